# revision 29
# baseline (speedup 1.0000x reference)
"""DeepStitch Trainium2 kernel.

Problem (per batch b of 4):
  resp = sum_c feature_A[b]            -> [128,128]
  16x16 adaptive max-pool (8x8 windows) with argmax -> 256 keypoint positions
  desc = feature_A[b][:, keypoints]    -> [512, 256]
  dist[k, n] = ||desc_k - feature_B[b][:, n]||^2 over all 16384 positions
  min/argmin over n -> min_vals, matched positions
  mode vote over (drow, dcol) -> offsets[b]

Sharding: 8 cores = 4 batches x 2 keypoint-halves. Each core:
  - streams its half's window rows of A^T (j-major layout) and computes resp
    via ScalarE accumulate, pools with max/max_index, gathers its 128
    descriptors by indirect DMA,
  - computes negdist = 2*desc@B - bn over the FULL 16384 positions for its
    128 keypoints with a bf16 hi/lo 3-term matmul (fp32-class accuracy),
  - reduces with max/max_index to the per-keypoint min distance + argmin.
Host does the sharding/layout prep, the [B,256]->[B,2] mode vote and the
final assembly (all trivial glue).
"""
import numpy as np
import ml_dtypes

import concourse.bacc as bacc
import concourse.tile as tile
import concourse.mybir as mybir
from concourse.bass import IndirectOffsetOnAxis, ts
from concourse.bass_utils import run_bass_kernel_spmd

B, C, H, W = 4, 512, 128, 128
S = 16          # pooling grid
BH = H // S     # 8 (window side)
K = 256         # keypoints per batch
KH = 128        # keypoints per core (k-half)
HW = H * W      # 16384
NT = 512        # n tile
NTILES = HW // NT      # 32
JT = 64         # positions per window
CCH = C // 128  # 4 contraction chunks

f32 = mybir.dt.float32
bf16 = mybir.dt.bfloat16
u32 = mybir.dt.uint32
ActF = mybir.ActivationFunctionType
Alu = mybir.AluOpType

IOTA = np.arange(128, dtype=np.uint32).reshape(128, 1)
IDN = np.eye(128, dtype=np.float32)

_CACHE = {}
LAST_EXEC_NS = None


def _build():
    nc = bacc.Bacc("TRN2", target_bir_lowering=False, debug=False)
    fat_d = nc.dram_tensor("fat", [KH * JT, C], f32, kind="ExternalInput").ap()
    fbh_d = nc.dram_tensor("fbh", [C, HW], bf16, kind="ExternalInput").ap()
    fbl_d = nc.dram_tensor("fbl", [C, HW], bf16, kind="ExternalInput").ap()
    bns_d = nc.dram_tensor("bns", [3, HW], bf16, kind="ExternalInput").ap()
    iota_d = nc.dram_tensor("iota", [128, 1], u32, kind="ExternalInput").ap()
    idn_d = nc.dram_tensor("idn", [128, 128], f32, kind="ExternalInput").ap()
    non_d = nc.dram_tensor("non", [3, 128], bf16, kind="ExternalInput").ap()
    o_loc = nc.dram_tensor("o_loc", [128, 1], u32, kind="ExternalOutput").ap()
    o_dn = nc.dram_tensor("o_dn", [128, 1], f32, kind="ExternalOutput").ap()
    o_mx = nc.dram_tensor("o_mx", [128, 1], f32, kind="ExternalOutput").ap()
    o_mi = nc.dram_tensor("o_mi", [128, 1], u32, kind="ExternalOutput").ap()

    with tile.TileContext(nc) as tc:
        with tc.tile_pool(name="const", bufs=1) as const, \
             tc.tile_pool(name="fatp", bufs=3) as fat_p, \
             tc.tile_pool(name="fbp", bufs=8) as fb_p, \
             tc.tile_pool(name="work", bufs=1) as work, \
             tc.tile_pool(name="ev", bufs=2) as ev_p, \
             tc.tile_pool(name="psx", bufs=6, space="PSUM") as ps_x:
            iota_t = const.tile([128, 1], u32)
            nc.sync.dma_start(out=iota_t[:], in_=iota_d)
            idn_t = const.tile([128, 128], f32)
            nc.sync.dma_start(out=idn_t[:], in_=idn_d)
            non_t = const.tile([3, 128], bf16)
            nc.sync.dma_start(out=non_t[:], in_=non_d)
            bns_t = const.tile([3, HW], bf16)
            nc.sync.dma_start(out=bns_t[:], in_=bns_d)

            # ---- phase A: resp accumulate over this core's 128 windows ----
            # fat row m = j*128 + kloc  (j in [0,64), kloc in [0,128))
            resp_sb = work.tile([128, JT], f32)
            for t4 in range(JT // 4):      # 16 chunks of [512 rows, 512]
                ft = fat_p.tile([128, 4, C], f32, tag="fat")
                nc.sync.dma_start(
                    out=ft[:],
                    in_=fat_d[ts(t4, 512), :].rearrange("(tt p) c -> p tt c", p=128))
                for tt in range(4):
                    t = t4 * 4 + tt
                    if tt % 2 == 0:
                        sc = fat_p.tile([128, C], f32, tag="fatsq")
                        nc.scalar.activation(sc[:], ft[:, tt, :], ActF.Copy,
                                             accum_out=resp_sb[:, t:t + 1])
                    else:
                        nc.vector.tensor_reduce(
                            out=resp_sb[:, t:t + 1], in_=ft[:, tt, :],
                            axis=mybir.AxisListType.X, op=Alu.add)

            # ---- pooling: per-window argmax over j ----
            vmax = ev_p.tile([128, 8], f32, tag="vmax")
            nc.vector.max(out=vmax[:], in_=resp_sb[:])
            ju = ev_p.tile([128, 8], u32, tag="ju")
            nc.vector.max_index(out=ju[:], in_max=vmax[:], in_values=resp_sb[:])
            loc_t = work.tile([128, 1], u32)
            nc.vector.tensor_copy(loc_t[:], ju[:, 0:1])
            mi_t = work.tile([128, 1], u32)
            nc.vector.tensor_scalar(out=mi_t[:], in0=ju[:, 0:1],
                                    scalar1=128, scalar2=None, op0=Alu.mult)
            nc.vector.tensor_add(mi_t[:], mi_t[:], iota_t[:])

            # ---- descriptor gather + dn + hi/lo split (x2 scale) ----
            desc_t = work.tile([128, C], f32)
            nc.gpsimd.indirect_dma_start(
                out=desc_t[:], out_offset=None,
                in_=fat_d, in_offset=IndirectOffsetOnAxis(ap=mi_t[:, :1], axis=0))
            dsq = work.tile([128, C], f32)
            dn_t = work.tile([128, 1], f32)
            nc.scalar.activation(dsq[:], desc_t[:], ActF.Square, accum_out=dn_t[:])
            d2h = work.tile([128, CCH, 128], bf16)
            d2l = work.tile([128, CCH, 128], bf16)
            for cc in range(CCH):
                pt = ps_x.tile([128, 128], f32, tag="pst", bufs=2,
                               name=f"pt_{cc}")
                nc.tensor.transpose(pt[:], desc_t[:, ts(cc, 128)], idn_t[:])
                nc.vector.tensor_scalar_mul(d2h[:, cc, :], pt[:], 2.0)
                nc.vector.scalar_tensor_tensor(
                    out=d2l[:, cc, :], in0=pt[:], scalar=2.0,
                    in1=d2h[:, cc, :], op0=Alu.mult, op1=Alu.subtract)
            nc.sync.dma_start(out=o_loc, in_=loc_t[:])
            nc.sync.dma_start(out=o_dn, in_=dn_t[:])

            # ---- phase B: negdist = 2*desc@fb - bn, streamed over n ----
            # bn subtracted via K=1 matmuls of -ones with a 3-term bf16 split
            negdist = work.tile([128, HW], f32)
            maxparts = work.tile([128, NTILES, 8], f32)
            fbh_v = fbh_d.rearrange("(cc p) n -> p cc n", cc=CCH)
            fbl_v = fbl_d.rearrange("(cc p) n -> p cc n", cc=CCH)
            CHUNK = 8                      # nt tiles per argmax chunk
            mx_t = work.tile([128, 1], f32)
            mi_out = work.tile([128, 1], u32)
            NG = 4                         # n tiles per weight-reuse group
            for ng in range(NTILES // NG):
                fhs, fls, pxs = [], [], []
                for g in range(NG):
                    nt = ng * NG + g
                    fh = fb_p.tile([128, CCH, NT], bf16, tag="fbh",
                                   name=f"fh_{nt}")
                    nc.sync.dma_start(out=fh[:], in_=fbh_v[:, :, ts(nt, NT)])
                    fl = fb_p.tile([128, CCH, NT], bf16, tag="fbl",
                                   name=f"fl_{nt}")
                    nc.sync.dma_start(out=fl[:], in_=fbl_v[:, :, ts(nt, NT)])
                    fhs.append(fh)
                    fls.append(fl)
                    pxs.append(ps_x.tile([128, NT], f32, tag="psx",
                                         name=f"px_{nt}"))
                for cc in range(CCH):
                    for g in range(NG):
                        nc.tensor.matmul(pxs[g][:], d2h[:, cc, :],
                                         fhs[g][:, cc, :],
                                         start=(cc == 0), stop=False)
                for cc in range(CCH):
                    for g in range(NG):
                        nc.tensor.matmul(pxs[g][:], d2h[:, cc, :],
                                         fls[g][:, cc, :],
                                         start=False, stop=False)
                for cc in range(CCH):
                    for g in range(NG):
                        nc.tensor.matmul(pxs[g][:], d2l[:, cc, :],
                                         fhs[g][:, cc, :],
                                         start=False, stop=False)
                for g in range(NG):
                    nt = ng * NG + g
                    nc.tensor.matmul(pxs[g][:], non_t[:], bns_t[:, ts(nt, NT)],
                                     start=False, stop=True)
                    nc.vector.tensor_copy(negdist[:, ts(nt, NT)], pxs[g][:])
                    nc.vector.max(out=maxparts[:, nt, :],
                                  in_=negdist[:, ts(nt, NT)])

                # ---- chunked argmax over finished spans (smaller tail) ----
                nt = ng * NG + NG - 1
                for c, (cs, span) in enumerate(
                        [(0, 8), (8, 8), (16, 8), (24, 4), (28, 4)]):
                    if nt + 1 != cs + span:
                        continue
                    gmax = ev_p.tile([128, 8], f32, tag="gmax")
                    nc.vector.max(out=gmax[:],
                                  in_=maxparts[:, cs:cs + span, :])
                    gmi = ev_p.tile([128, 8], u32, tag="gmi")
                    nc.vector.max_index(
                        out=gmi[:], in_max=gmax[:],
                        in_values=negdist[:, cs * NT:(cs + span) * NT])
                    if c == 0:
                        nc.vector.tensor_copy(mx_t[:], gmax[:, 0:1])
                        nc.vector.tensor_copy(mi_out[:], gmi[:, 0:1])
                    else:
                        gio = ev_p.tile([128, 1], u32, tag="gio")
                        nc.vector.tensor_scalar_add(gio[:], gmi[:, 0:1],
                                                    cs * NT)
                        pred = ev_p.tile([128, 1], u32, tag="pred")
                        nc.vector.tensor_tensor(out=pred[:], in0=gmax[:, 0:1],
                                                in1=mx_t[:], op=Alu.is_gt)
                        nc.vector.copy_predicated(mx_t[:], pred[:], gmax[:, 0:1])
                        nc.vector.copy_predicated(mi_out[:], pred[:], gio[:])

            nc.sync.dma_start(out=o_mx, in_=mx_t[:])
            nc.sync.dma_start(out=o_mi, in_=mi_out[:])
    nc.compile()
    return nc


def _get_nc():
    if "nc" not in _CACHE:
        _CACHE["nc"] = _build()
    return _CACHE["nc"]


def _prep_inputs(feature_A, feature_B):
    fa = np.ascontiguousarray(np.asarray(feature_A, dtype=np.float32))
    fb = np.asarray(feature_B, dtype=np.float32).reshape(B, C, HW)

    # fat: per (batch, khalf): rows m = j*128 + kloc, cols C.
    # window k = bi*16+bj (bi=k//16), j = r*8+s; pixel = (bi*8+r, bj*8+s)
    # fa [B,C,H,W] -> [B, C, bi(16), r(8), bj(16), s(8)]
    fa6 = fa.reshape(B, C, S, BH, S, BH)
    # -> [B, khalf(2), klo_bi(8), bj(16), r(8), s(8), C] with k-half on bi
    fat = fa6.transpose(0, 2, 4, 3, 5, 1).reshape(B, 2, 8, 16, BH, BH, C)
    # rows must be ordered (j=(r,s) major, then kloc=(bi_lo, bj)):
    # -> [B, 2, r, s, bi_lo, bj, C]
    fat = fat.transpose(0, 1, 4, 5, 2, 3, 6).reshape(B, 2, KH * JT, C)
    fat = np.ascontiguousarray(fat)

    fbh = fb.astype(ml_dtypes.bfloat16)
    fbl = (fb - fbh.astype(np.float32)).astype(ml_dtypes.bfloat16)
    bn = (fb.astype(np.float64) ** 2).sum(axis=1, dtype=np.float64)
    bn = bn.astype(np.float32)                      # [B, HW]
    bnh = bn.astype(ml_dtypes.bfloat16)
    r1 = bn - bnh.astype(np.float32)
    bnl = r1.astype(ml_dtypes.bfloat16)
    bnl2 = (r1 - bnl.astype(np.float32)).astype(ml_dtypes.bfloat16)
    bns = np.stack([bnh, bnl, bnl2], axis=1)        # [B, 3, HW]
    non = -np.ones((3, 128), dtype=ml_dtypes.bfloat16)

    in_maps = []
    for core in range(8):
        b, h = core // 2, core % 2
        in_maps.append({
            "fat": fat[b, h],
            "fbh": fbh[b],
            "fbl": fbl[b],
            "bns": bns[b],
            "non": non,
            "iota": IOTA,
            "idn": IDN,
        })
    return in_maps


def _decode(results):
    offsets = np.zeros((B, 2), dtype=np.int32)
    min_vals = np.zeros((B, K), dtype=np.float32)
    for b in range(B):
        r0, r1 = results[2 * b], results[2 * b + 1]
        loc = np.concatenate([r0["o_loc"][:, 0], r1["o_loc"][:, 0]]).astype(np.int64)
        dn = np.concatenate([r0["o_dn"][:, 0], r1["o_dn"][:, 0]])
        mx = np.concatenate([r0["o_mx"][:, 0], r1["o_mx"][:, 0]])
        mi = np.concatenate([r0["o_mi"][:, 0], r1["o_mi"][:, 0]]).astype(np.int64)

        # loc is resp_sb free index = j then kloc... ju indexes the free dim of
        # resp_sb[128, 64]: value is j directly (free dim is j).
        k = np.arange(K)
        bi, bj = k // S, k % S
        r_, s_ = loc // BH, loc % BH
        row_A = bi * BH + r_
        col_A = bj * BH + s_

        min_vals[b] = dn - mx
        row_B = mi // W
        col_B = mi % W
        drow = row_A - row_B
        dcol = col_A - col_B
        code = drow * (4 * W) + dcol
        counts = (code[:, None] == code[None, :]).sum(-1)
        best = int(np.argmax(counts))
        offsets[b, 0] = drow[best]
        offsets[b, 1] = dcol[best]
    return offsets, min_vals


def kernel(feature_A, feature_B, trace=False):
    global LAST_EXEC_NS
    nc = _get_nc()
    in_maps = _prep_inputs(feature_A, feature_B)
    try:
        res = run_bass_kernel_spmd(nc, in_maps, core_ids=list(range(8)), trace=trace)
    except Exception:
        if not trace:
            raise
        res = run_bass_kernel_spmd(nc, in_maps, core_ids=list(range(8)), trace=False)
    LAST_EXEC_NS = getattr(res, "exec_time_ns", None)
    return _decode(res.results)


# revision 36
# speedup vs baseline: 1.0955x; 1.0955x over previous
"""DeepStitch Trainium2 kernel.

Problem (per batch b of 4):
  resp = sum_c feature_A[b]            -> [128,128]
  16x16 adaptive max-pool (8x8 windows) with argmax -> 256 keypoint positions
  desc = feature_A[b][:, keypoints]    -> [512, 256]
  dist[k, n] = ||desc_k - feature_B[b][:, n]||^2 over all 16384 positions
  min/argmin over n -> min_vals, matched positions
  mode vote over (drow, dcol) -> offsets[b]

Sharding: 8 cores = 4 batches x 2 keypoint-halves. Each core:
  - streams its half's window rows of A^T (j-major layout) and computes resp
    via ScalarE accumulate, pools with max/max_index, gathers its 128
    descriptors by indirect DMA,
  - computes negdist = 2*desc@B - bn over the FULL 16384 positions for its
    128 keypoints with a bf16 hi/lo 3-term matmul (fp32-class accuracy),
  - reduces with max/max_index to the per-keypoint min distance + argmin.
Host does the sharding/layout prep, the [B,256]->[B,2] mode vote and the
final assembly (all trivial glue).
"""
import numpy as np
import ml_dtypes

import concourse.bacc as bacc
import concourse.tile as tile
import concourse.mybir as mybir
from concourse.bass import IndirectOffsetOnAxis, ts
from concourse.bass_utils import run_bass_kernel_spmd

B, C, H, W = 4, 512, 128, 128
S = 16          # pooling grid
BH = H // S     # 8 (window side)
K = 256         # keypoints per batch
KH = 128        # keypoints per core (k-half)
HW = H * W      # 16384
NT = 512        # n tile
NTILES = HW // NT      # 32
JT = 64         # positions per window
CCH = C // 128  # 4 contraction chunks

f32 = mybir.dt.float32
bf16 = mybir.dt.bfloat16
u32 = mybir.dt.uint32
ActF = mybir.ActivationFunctionType
Alu = mybir.AluOpType

IOTA = np.arange(128, dtype=np.uint32).reshape(128, 1)
IDN = np.eye(128, dtype=np.float32)

_CACHE = {}
LAST_EXEC_NS = None


def _build():
    nc = bacc.Bacc("TRN2", target_bir_lowering=False, debug=False)
    fat_d = nc.dram_tensor("fat", [KH * JT, C], f32, kind="ExternalInput").ap()
    fbh_d = nc.dram_tensor("fbh", [C, HW], bf16, kind="ExternalInput").ap()
    fbl_d = nc.dram_tensor("fbl", [C, HW], bf16, kind="ExternalInput").ap()
    bns_d = nc.dram_tensor("bns", [3, HW], bf16, kind="ExternalInput").ap()
    iota_d = nc.dram_tensor("iota", [128, 1], u32, kind="ExternalInput").ap()
    idn_d = nc.dram_tensor("idn", [128, 128], f32, kind="ExternalInput").ap()
    non_d = nc.dram_tensor("non", [3, 128], bf16, kind="ExternalInput").ap()
    o_loc = nc.dram_tensor("o_loc", [128, 1], u32, kind="ExternalOutput").ap()
    o_dn = nc.dram_tensor("o_dn", [128, 1], f32, kind="ExternalOutput").ap()
    o_mx = nc.dram_tensor("o_mx", [128, 1], f32, kind="ExternalOutput").ap()
    o_mi = nc.dram_tensor("o_mi", [128, 1], u32, kind="ExternalOutput").ap()

    with tile.TileContext(nc) as tc:
        with tc.tile_pool(name="const", bufs=1) as const, \
             tc.tile_pool(name="fatp", bufs=3) as fat_p, \
             tc.tile_pool(name="fbp", bufs=8) as fb_p, \
             tc.tile_pool(name="work", bufs=1) as work, \
             tc.tile_pool(name="ev", bufs=2) as ev_p, \
             tc.tile_pool(name="psx", bufs=6, space="PSUM") as ps_x:
            iota_t = const.tile([128, 1], u32)
            nc.sync.dma_start(out=iota_t[:], in_=iota_d)
            idn_t = const.tile([128, 128], f32)
            nc.sync.dma_start(out=idn_t[:], in_=idn_d)
            non_t = const.tile([3, 128], bf16)
            nc.sync.dma_start(out=non_t[:], in_=non_d)
            bns_t = const.tile([3, HW], bf16)
            nc.sync.dma_start(out=bns_t[:], in_=bns_d)

            # ---- phase A: resp accumulate over this core's 128 windows ----
            # fat row m = j*128 + kloc  (j in [0,64), kloc in [0,128))
            resp_sb = work.tile([128, JT], f32)
            for t4 in range(JT // 4):      # 16 chunks of [512 rows, 512]
                ft = fat_p.tile([128, 4, C], f32, tag="fat")
                nc.sync.dma_start(
                    out=ft[:],
                    in_=fat_d[ts(t4, 512), :].rearrange("(tt p) c -> p tt c", p=128))
                for tt in range(4):
                    t = t4 * 4 + tt
                    if tt % 2 == 0:
                        sc = fat_p.tile([128, C], f32, tag="fatsq")
                        nc.scalar.activation(sc[:], ft[:, tt, :], ActF.Copy,
                                             accum_out=resp_sb[:, t:t + 1])
                    else:
                        nc.vector.tensor_reduce(
                            out=resp_sb[:, t:t + 1], in_=ft[:, tt, :],
                            axis=mybir.AxisListType.X, op=Alu.add)

            # ---- pooling: per-window argmax over j ----
            vmax = ev_p.tile([128, 8], f32, tag="vmax")
            nc.vector.max(out=vmax[:], in_=resp_sb[:])
            ju = ev_p.tile([128, 8], u32, tag="ju")
            nc.vector.max_index(out=ju[:], in_max=vmax[:], in_values=resp_sb[:])
            loc_t = work.tile([128, 1], u32)
            nc.vector.tensor_copy(loc_t[:], ju[:, 0:1])
            mi_t = work.tile([128, 1], u32)
            nc.vector.tensor_scalar(out=mi_t[:], in0=ju[:, 0:1],
                                    scalar1=128, scalar2=None, op0=Alu.mult)
            nc.vector.tensor_add(mi_t[:], mi_t[:], iota_t[:])

            # ---- descriptor gather + dn + hi/lo split (x2 scale) ----
            desc_t = work.tile([128, C], f32)
            nc.gpsimd.indirect_dma_start(
                out=desc_t[:], out_offset=None,
                in_=fat_d, in_offset=IndirectOffsetOnAxis(ap=mi_t[:, :1], axis=0))
            dsq = work.tile([128, C], f32)
            dn_t = work.tile([128, 1], f32)
            nc.scalar.activation(dsq[:], desc_t[:], ActF.Square, accum_out=dn_t[:])
            d2h = work.tile([128, CCH, 128], bf16)
            d2l = work.tile([128, CCH, 128], bf16)
            for cc in range(CCH):
                pt = ps_x.tile([128, 128], f32, tag="pst", bufs=2,
                               name=f"pt_{cc}")
                nc.tensor.transpose(pt[:], desc_t[:, ts(cc, 128)], idn_t[:])
                nc.vector.tensor_scalar_mul(d2h[:, cc, :], pt[:], 2.0)
                nc.vector.scalar_tensor_tensor(
                    out=d2l[:, cc, :], in0=pt[:], scalar=2.0,
                    in1=d2h[:, cc, :], op0=Alu.mult, op1=Alu.subtract)

            # ---- phase B: negdist = 2*desc@fb - bn, streamed over n ----
            # bn subtracted via K=1 matmuls of -ones with a 3-term bf16 split
            negdist = work.tile([128, HW], f32)
            maxparts = work.tile([128, NTILES, 8], f32)
            fbh_v = fbh_d.rearrange("(cc p) n -> p cc n", cc=CCH)
            fbl_v = fbl_d.rearrange("(cc p) n -> p cc n", cc=CCH)
            CHUNK = 8                      # nt tiles per argmax chunk
            mx_t = work.tile([128, 1], f32)
            mi_out = work.tile([128, 1], u32)
            NG = 4                         # n tiles per weight-reuse group
            for ng in range(NTILES // NG):
                fhs, fls, pxs = [], [], []
                for g in range(NG):
                    nt = ng * NG + g
                    fh = fb_p.tile([128, CCH, NT], bf16, tag="fbh",
                                   name=f"fh_{nt}")
                    nc.sync.dma_start(out=fh[:], in_=fbh_v[:, :, ts(nt, NT)])
                    fl = fb_p.tile([128, CCH, NT], bf16, tag="fbl",
                                   name=f"fl_{nt}")
                    nc.sync.dma_start(out=fl[:], in_=fbl_v[:, :, ts(nt, NT)])
                    fhs.append(fh)
                    fls.append(fl)
                    pxs.append(ps_x.tile([128, NT], f32, tag="psx",
                                         name=f"px_{nt}"))
                for cc in range(CCH):
                    for g in range(NG):
                        nc.tensor.matmul(pxs[g][:], d2h[:, cc, :],
                                         fhs[g][:, cc, :],
                                         start=(cc == 0), stop=False)
                for cc in range(CCH):
                    for g in range(NG):
                        nc.tensor.matmul(pxs[g][:], d2h[:, cc, :],
                                         fls[g][:, cc, :],
                                         start=False, stop=False)
                for cc in range(CCH):
                    for g in range(NG):
                        nc.tensor.matmul(pxs[g][:], d2l[:, cc, :],
                                         fhs[g][:, cc, :],
                                         start=False, stop=False)
                for g in range(NG):
                    nt = ng * NG + g
                    nc.tensor.matmul(pxs[g][:], non_t[:], bns_t[:, ts(nt, NT)],
                                     start=False, stop=True)
                    nc.scalar.activation(negdist[:, ts(nt, NT)], pxs[g][:],
                                         ActF.Copy)
                    nc.vector.max(out=maxparts[:, nt, :],
                                  in_=negdist[:, ts(nt, NT)])

                # ---- chunked argmax over finished 4096-wide spans ----
                nt = ng * NG + NG - 1
                if (nt + 1) % CHUNK == 0:
                    c = nt // CHUNK
                    gmax = ev_p.tile([128, 8], f32, tag="gmax")
                    nc.vector.max(out=gmax[:],
                                  in_=maxparts[:, ts(c, CHUNK), :])
                    gmi = ev_p.tile([128, 8], u32, tag="gmi")
                    nc.vector.max_index(
                        out=gmi[:], in_max=gmax[:],
                        in_values=negdist[:, ts(c, CHUNK * NT)])
                    if c == 0:
                        nc.vector.tensor_copy(mx_t[:], gmax[:, 0:1])
                        nc.vector.tensor_copy(mi_out[:], gmi[:, 0:1])
                    else:
                        gio = ev_p.tile([128, 1], u32, tag="gio")
                        nc.vector.tensor_scalar_add(gio[:], gmi[:, 0:1],
                                                    c * CHUNK * NT)
                        pred = ev_p.tile([128, 1], u32, tag="pred")
                        nc.vector.tensor_tensor(out=pred[:], in0=gmax[:, 0:1],
                                                in1=mx_t[:], op=Alu.is_gt)
                        nc.vector.copy_predicated(mx_t[:], pred[:], gmax[:, 0:1])
                        nc.vector.copy_predicated(mi_out[:], pred[:], gio[:])

            nc.sync.dma_start(out=o_loc, in_=loc_t[:])
            nc.sync.dma_start(out=o_dn, in_=dn_t[:])
            nc.sync.dma_start(out=o_mx, in_=mx_t[:])
            nc.sync.dma_start(out=o_mi, in_=mi_out[:])
    nc.compile()
    return nc


def _get_nc():
    if "nc" not in _CACHE:
        _CACHE["nc"] = _build()
    return _CACHE["nc"]


def _prep_inputs(feature_A, feature_B):
    fa = np.ascontiguousarray(np.asarray(feature_A, dtype=np.float32))
    fb = np.asarray(feature_B, dtype=np.float32).reshape(B, C, HW)

    # fat: per (batch, khalf): rows m = j*128 + kloc, cols C.
    # window k = bi*16+bj (bi=k//16), j = r*8+s; pixel = (bi*8+r, bj*8+s)
    # fa [B,C,H,W] -> [B, C, bi(16), r(8), bj(16), s(8)]
    fa6 = fa.reshape(B, C, S, BH, S, BH)
    # -> [B, khalf(2), klo_bi(8), bj(16), r(8), s(8), C] with k-half on bi
    fat = fa6.transpose(0, 2, 4, 3, 5, 1).reshape(B, 2, 8, 16, BH, BH, C)
    # rows must be ordered (j=(r,s) major, then kloc=(bi_lo, bj)):
    # -> [B, 2, r, s, bi_lo, bj, C]
    fat = fat.transpose(0, 1, 4, 5, 2, 3, 6).reshape(B, 2, KH * JT, C)
    fat = np.ascontiguousarray(fat)

    fbh = fb.astype(ml_dtypes.bfloat16)
    fbl = (fb - fbh.astype(np.float32)).astype(ml_dtypes.bfloat16)
    bn = (fb.astype(np.float64) ** 2).sum(axis=1, dtype=np.float64)
    bn = bn.astype(np.float32)                      # [B, HW]
    bnh = bn.astype(ml_dtypes.bfloat16)
    r1 = bn - bnh.astype(np.float32)
    bnl = r1.astype(ml_dtypes.bfloat16)
    bnl2 = (r1 - bnl.astype(np.float32)).astype(ml_dtypes.bfloat16)
    bns = np.stack([bnh, bnl, bnl2], axis=1)        # [B, 3, HW]
    non = -np.ones((3, 128), dtype=ml_dtypes.bfloat16)

    in_maps = []
    for core in range(8):
        b, h = core // 2, core % 2
        in_maps.append({
            "fat": fat[b, h],
            "fbh": fbh[b],
            "fbl": fbl[b],
            "bns": bns[b],
            "non": non,
            "iota": IOTA,
            "idn": IDN,
        })
    return in_maps


def _decode(results):
    offsets = np.zeros((B, 2), dtype=np.int32)
    min_vals = np.zeros((B, K), dtype=np.float32)
    for b in range(B):
        r0, r1 = results[2 * b], results[2 * b + 1]
        loc = np.concatenate([r0["o_loc"][:, 0], r1["o_loc"][:, 0]]).astype(np.int64)
        dn = np.concatenate([r0["o_dn"][:, 0], r1["o_dn"][:, 0]])
        mx = np.concatenate([r0["o_mx"][:, 0], r1["o_mx"][:, 0]])
        mi = np.concatenate([r0["o_mi"][:, 0], r1["o_mi"][:, 0]]).astype(np.int64)

        # loc is resp_sb free index = j then kloc... ju indexes the free dim of
        # resp_sb[128, 64]: value is j directly (free dim is j).
        k = np.arange(K)
        bi, bj = k // S, k % S
        r_, s_ = loc // BH, loc % BH
        row_A = bi * BH + r_
        col_A = bj * BH + s_

        min_vals[b] = dn - mx
        row_B = mi // W
        col_B = mi % W
        drow = row_A - row_B
        dcol = col_A - col_B
        code = drow * (4 * W) + dcol
        counts = (code[:, None] == code[None, :]).sum(-1)
        best = int(np.argmax(counts))
        offsets[b, 0] = drow[best]
        offsets[b, 1] = dcol[best]
    return offsets, min_vals


def kernel(feature_A, feature_B, trace=False):
    global LAST_EXEC_NS
    nc = _get_nc()
    in_maps = _prep_inputs(feature_A, feature_B)
    try:
        res = run_bass_kernel_spmd(nc, in_maps, core_ids=list(range(8)), trace=trace)
    except Exception:
        if not trace:
            raise
        res = run_bass_kernel_spmd(nc, in_maps, core_ids=list(range(8)), trace=False)
    LAST_EXEC_NS = getattr(res, "exec_time_ns", None)
    return _decode(res.results)


# revision 37
# speedup vs baseline: 1.1383x; 1.0391x over previous
"""DeepStitch Trainium2 kernel.

Problem (per batch b of 4):
  resp = sum_c feature_A[b]            -> [128,128]
  16x16 adaptive max-pool (8x8 windows) with argmax -> 256 keypoint positions
  desc = feature_A[b][:, keypoints]    -> [512, 256]
  dist[k, n] = ||desc_k - feature_B[b][:, n]||^2 over all 16384 positions
  min/argmin over n -> min_vals, matched positions
  mode vote over (drow, dcol) -> offsets[b]

Sharding: 8 cores = 4 batches x 2 keypoint-halves. Each core:
  - streams its half's window rows of A^T (j-major layout) and computes resp
    via ScalarE accumulate, pools with max/max_index, gathers its 128
    descriptors by indirect DMA,
  - computes negdist = 2*desc@B - bn over the FULL 16384 positions for its
    128 keypoints with a bf16 hi/lo 3-term matmul (fp32-class accuracy),
  - reduces with max/max_index to the per-keypoint min distance + argmin.
Host does the sharding/layout prep, the [B,256]->[B,2] mode vote and the
final assembly (all trivial glue).
"""
import numpy as np
import ml_dtypes

import concourse.bacc as bacc
import concourse.tile as tile
import concourse.mybir as mybir
from concourse.bass import IndirectOffsetOnAxis, ts
from concourse.bass_utils import run_bass_kernel_spmd

B, C, H, W = 4, 512, 128, 128
S = 16          # pooling grid
BH = H // S     # 8 (window side)
K = 256         # keypoints per batch
KH = 128        # keypoints per core (k-half)
HW = H * W      # 16384
NT = 512        # n tile
NTILES = HW // NT      # 32
JT = 64         # positions per window
CCH = C // 128  # 4 contraction chunks

f32 = mybir.dt.float32
bf16 = mybir.dt.bfloat16
u32 = mybir.dt.uint32
ActF = mybir.ActivationFunctionType
Alu = mybir.AluOpType

IOTA = np.arange(128, dtype=np.uint32).reshape(128, 1)
IDN = np.eye(128, dtype=np.float32)

_CACHE = {}
LAST_EXEC_NS = None


def _build():
    nc = bacc.Bacc("TRN2", target_bir_lowering=False, debug=False)
    fat_d = nc.dram_tensor("fat", [KH * JT, C], f32, kind="ExternalInput").ap()
    fbh_d = nc.dram_tensor("fbh", [C, HW], bf16, kind="ExternalInput").ap()
    fbl_d = nc.dram_tensor("fbl", [C, HW], bf16, kind="ExternalInput").ap()
    bns_d = nc.dram_tensor("bns", [3, HW], bf16, kind="ExternalInput").ap()
    iota_d = nc.dram_tensor("iota", [128, 1], u32, kind="ExternalInput").ap()
    idn_d = nc.dram_tensor("idn", [128, 128], f32, kind="ExternalInput").ap()
    non_d = nc.dram_tensor("non", [3, 128], bf16, kind="ExternalInput").ap()
    o_loc = nc.dram_tensor("o_loc", [128, 1], u32, kind="ExternalOutput").ap()
    o_dn = nc.dram_tensor("o_dn", [128, 1], f32, kind="ExternalOutput").ap()
    o_mx = nc.dram_tensor("o_mx", [128, 1], f32, kind="ExternalOutput").ap()
    o_mi = nc.dram_tensor("o_mi", [128, 1], u32, kind="ExternalOutput").ap()

    with tile.TileContext(nc) as tc:
        with tc.tile_pool(name="const", bufs=1) as const, \
             tc.tile_pool(name="fatp", bufs=3) as fat_p, \
             tc.tile_pool(name="fbp", bufs=8) as fb_p, \
             tc.tile_pool(name="work", bufs=1) as work, \
             tc.tile_pool(name="ev", bufs=2) as ev_p, \
             tc.tile_pool(name="psx", bufs=6, space="PSUM") as ps_x:
            iota_t = const.tile([128, 1], u32)
            nc.sync.dma_start(out=iota_t[:], in_=iota_d)
            idn_t = const.tile([128, 128], f32)
            nc.sync.dma_start(out=idn_t[:], in_=idn_d)
            non_t = const.tile([3, 128], bf16)
            nc.sync.dma_start(out=non_t[:], in_=non_d)
            bns_t = const.tile([3, HW], bf16)
            nc.sync.dma_start(out=bns_t[:], in_=bns_d)

            # ---- phase A: resp accumulate over this core's 128 windows ----
            # fat row m = j*128 + kloc  (j in [0,64), kloc in [0,128))
            resp_sb = work.tile([128, JT], f32)
            for t4 in range(JT // 4):      # 16 chunks of [512 rows, 512]
                ft = fat_p.tile([128, 4, C], f32, tag="fat")
                nc.sync.dma_start(
                    out=ft[:],
                    in_=fat_d[ts(t4, 512), :].rearrange("(tt p) c -> p tt c", p=128))
                for tt in range(4):
                    t = t4 * 4 + tt
                    if tt % 2 == 0:
                        sc = fat_p.tile([128, C], f32, tag="fatsq")
                        nc.scalar.activation(sc[:], ft[:, tt, :], ActF.Copy,
                                             accum_out=resp_sb[:, t:t + 1])
                    else:
                        nc.vector.tensor_reduce(
                            out=resp_sb[:, t:t + 1], in_=ft[:, tt, :],
                            axis=mybir.AxisListType.X, op=Alu.add)

            # ---- pooling: per-window argmax over j ----
            vmax = ev_p.tile([128, 8], f32, tag="vmax")
            nc.vector.max(out=vmax[:], in_=resp_sb[:])
            ju = ev_p.tile([128, 8], u32, tag="ju")
            nc.vector.max_index(out=ju[:], in_max=vmax[:], in_values=resp_sb[:])
            loc_t = work.tile([128, 1], u32)
            nc.vector.tensor_copy(loc_t[:], ju[:, 0:1])
            mi_t = work.tile([128, 1], u32)
            nc.vector.tensor_scalar(out=mi_t[:], in0=ju[:, 0:1],
                                    scalar1=128, scalar2=None, op0=Alu.mult)
            nc.vector.tensor_add(mi_t[:], mi_t[:], iota_t[:])

            # ---- descriptor gather + dn + hi/lo split (x2 scale) ----
            desc_t = work.tile([128, C], f32)
            nc.gpsimd.indirect_dma_start(
                out=desc_t[:], out_offset=None,
                in_=fat_d, in_offset=IndirectOffsetOnAxis(ap=mi_t[:, :1], axis=0))
            dsq = work.tile([128, C], f32)
            dn_t = work.tile([128, 1], f32)
            nc.scalar.activation(dsq[:], desc_t[:], ActF.Square, accum_out=dn_t[:])
            d2h = work.tile([128, CCH, 128], bf16)
            d2l = work.tile([128, CCH, 128], bf16)
            for cc in range(CCH):
                pt = ps_x.tile([128, 128], f32, tag="pst", bufs=2,
                               name=f"pt_{cc}")
                nc.tensor.transpose(pt[:], desc_t[:, ts(cc, 128)], idn_t[:])
                nc.vector.tensor_scalar_mul(d2h[:, cc, :], pt[:], 2.0)
                nc.vector.scalar_tensor_tensor(
                    out=d2l[:, cc, :], in0=pt[:], scalar=2.0,
                    in1=d2h[:, cc, :], op0=Alu.mult, op1=Alu.subtract)

            # ---- phase B: negdist = 2*desc@fb - bn, streamed over n ----
            # bn subtracted via K=1 matmuls of -ones with a 3-term bf16 split
            negdist = work.tile([128, HW], f32)
            maxparts = work.tile([128, NTILES, 8], f32)
            fbh_v = fbh_d.rearrange("(cc p) n -> p cc n", cc=CCH)
            fbl_v = fbl_d.rearrange("(cc p) n -> p cc n", cc=CCH)
            CHUNK = 8                      # nt tiles per argmax chunk
            mx_t = work.tile([128, 1], f32)
            mi_out = work.tile([128, 1], u32)
            NG = 4                         # n tiles per weight-reuse group
            for ng in range(NTILES // NG):
                fhs, fls, pxs = [], [], []
                for g in range(NG):
                    nt = ng * NG + g
                    fh = fb_p.tile([128, CCH, NT], bf16, tag="fbh",
                                   name=f"fh_{nt}")
                    nc.sync.dma_start(out=fh[:], in_=fbh_v[:, :, ts(nt, NT)])
                    fl = fb_p.tile([128, CCH, NT], bf16, tag="fbl",
                                   name=f"fl_{nt}")
                    nc.sync.dma_start(out=fl[:], in_=fbl_v[:, :, ts(nt, NT)])
                    fhs.append(fh)
                    fls.append(fl)
                    pxs.append(ps_x.tile([128, NT], f32, tag="psx",
                                         name=f"px_{nt}"))
                for cc in range(CCH):
                    for g in range(NG):
                        nc.tensor.matmul(pxs[g][:], d2h[:, cc, :],
                                         fhs[g][:, cc, :],
                                         start=(cc == 0), stop=False)
                for cc in range(CCH):
                    for g in range(NG):
                        nc.tensor.matmul(pxs[g][:], d2h[:, cc, :],
                                         fls[g][:, cc, :],
                                         start=False, stop=False)
                for cc in range(CCH):
                    for g in range(NG):
                        nc.tensor.matmul(pxs[g][:], d2l[:, cc, :],
                                         fhs[g][:, cc, :],
                                         start=False, stop=False)
                for g in range(NG):
                    nt = ng * NG + g
                    nc.tensor.matmul(pxs[g][:], non_t[:], bns_t[:, ts(nt, NT)],
                                     start=False, stop=True)
                    nc.vector.tensor_copy(negdist[:, ts(nt, NT)], pxs[g][:])
                    nc.vector.max(out=maxparts[:, nt, :],
                                  in_=negdist[:, ts(nt, NT)])

                # ---- chunked argmax over finished 4096-wide spans ----
                nt = ng * NG + NG - 1
                if (nt + 1) % CHUNK == 0:
                    c = nt // CHUNK
                    gmax = ev_p.tile([128, 8], f32, tag="gmax")
                    nc.vector.max(out=gmax[:],
                                  in_=maxparts[:, ts(c, CHUNK), :])
                    gmi = ev_p.tile([128, 8], u32, tag="gmi")
                    nc.vector.max_index(
                        out=gmi[:], in_max=gmax[:],
                        in_values=negdist[:, ts(c, CHUNK * NT)])
                    if c == 0:
                        nc.vector.tensor_copy(mx_t[:], gmax[:, 0:1])
                        nc.vector.tensor_copy(mi_out[:], gmi[:, 0:1])
                    else:
                        gio = ev_p.tile([128, 1], u32, tag="gio")
                        nc.vector.tensor_scalar_add(gio[:], gmi[:, 0:1],
                                                    c * CHUNK * NT)
                        pred = ev_p.tile([128, 1], u32, tag="pred")
                        nc.vector.tensor_tensor(out=pred[:], in0=gmax[:, 0:1],
                                                in1=mx_t[:], op=Alu.is_gt)
                        nc.vector.copy_predicated(mx_t[:], pred[:], gmax[:, 0:1])
                        nc.vector.copy_predicated(mi_out[:], pred[:], gio[:])

            nc.sync.dma_start(out=o_loc, in_=loc_t[:])
            nc.sync.dma_start(out=o_dn, in_=dn_t[:])
            nc.sync.dma_start(out=o_mx, in_=mx_t[:])
            nc.sync.dma_start(out=o_mi, in_=mi_out[:])
    nc.compile()
    return nc


def _get_nc():
    if "nc" not in _CACHE:
        _CACHE["nc"] = _build()
    return _CACHE["nc"]


def _prep_inputs(feature_A, feature_B):
    fa = np.ascontiguousarray(np.asarray(feature_A, dtype=np.float32))
    fb = np.asarray(feature_B, dtype=np.float32).reshape(B, C, HW)

    # fat: per (batch, khalf): rows m = j*128 + kloc, cols C.
    # window k = bi*16+bj (bi=k//16), j = r*8+s; pixel = (bi*8+r, bj*8+s)
    # fa [B,C,H,W] -> [B, C, bi(16), r(8), bj(16), s(8)]
    fa6 = fa.reshape(B, C, S, BH, S, BH)
    # -> [B, khalf(2), klo_bi(8), bj(16), r(8), s(8), C] with k-half on bi
    fat = fa6.transpose(0, 2, 4, 3, 5, 1).reshape(B, 2, 8, 16, BH, BH, C)
    # rows must be ordered (j=(r,s) major, then kloc=(bi_lo, bj)):
    # -> [B, 2, r, s, bi_lo, bj, C]
    fat = fat.transpose(0, 1, 4, 5, 2, 3, 6).reshape(B, 2, KH * JT, C)
    fat = np.ascontiguousarray(fat)

    fbh = fb.astype(ml_dtypes.bfloat16)
    fbl = (fb - fbh.astype(np.float32)).astype(ml_dtypes.bfloat16)
    bn = (fb.astype(np.float64) ** 2).sum(axis=1, dtype=np.float64)
    bn = bn.astype(np.float32)                      # [B, HW]
    bnh = bn.astype(ml_dtypes.bfloat16)
    r1 = bn - bnh.astype(np.float32)
    bnl = r1.astype(ml_dtypes.bfloat16)
    bnl2 = (r1 - bnl.astype(np.float32)).astype(ml_dtypes.bfloat16)
    bns = np.stack([bnh, bnl, bnl2], axis=1)        # [B, 3, HW]
    non = -np.ones((3, 128), dtype=ml_dtypes.bfloat16)

    in_maps = []
    for core in range(8):
        b, h = core // 2, core % 2
        in_maps.append({
            "fat": fat[b, h],
            "fbh": fbh[b],
            "fbl": fbl[b],
            "bns": bns[b],
            "non": non,
            "iota": IOTA,
            "idn": IDN,
        })
    return in_maps


def _decode(results):
    offsets = np.zeros((B, 2), dtype=np.int32)
    min_vals = np.zeros((B, K), dtype=np.float32)
    for b in range(B):
        r0, r1 = results[2 * b], results[2 * b + 1]
        loc = np.concatenate([r0["o_loc"][:, 0], r1["o_loc"][:, 0]]).astype(np.int64)
        dn = np.concatenate([r0["o_dn"][:, 0], r1["o_dn"][:, 0]])
        mx = np.concatenate([r0["o_mx"][:, 0], r1["o_mx"][:, 0]])
        mi = np.concatenate([r0["o_mi"][:, 0], r1["o_mi"][:, 0]]).astype(np.int64)

        # loc is resp_sb free index = j then kloc... ju indexes the free dim of
        # resp_sb[128, 64]: value is j directly (free dim is j).
        k = np.arange(K)
        bi, bj = k // S, k % S
        r_, s_ = loc // BH, loc % BH
        row_A = bi * BH + r_
        col_A = bj * BH + s_

        min_vals[b] = dn - mx
        row_B = mi // W
        col_B = mi % W
        drow = row_A - row_B
        dcol = col_A - col_B
        code = drow * (4 * W) + dcol
        counts = (code[:, None] == code[None, :]).sum(-1)
        best = int(np.argmax(counts))
        offsets[b, 0] = drow[best]
        offsets[b, 1] = dcol[best]
    return offsets, min_vals


def kernel(feature_A, feature_B, trace=False):
    global LAST_EXEC_NS
    nc = _get_nc()
    in_maps = _prep_inputs(feature_A, feature_B)
    try:
        res = run_bass_kernel_spmd(nc, in_maps, core_ids=list(range(8)), trace=trace)
    except Exception:
        if not trace:
            raise
        res = run_bass_kernel_spmd(nc, in_maps, core_ids=list(range(8)), trace=False)
    LAST_EXEC_NS = getattr(res, "exec_time_ns", None)
    return _decode(res.results)
